# revision 4
# baseline (speedup 1.0000x reference)
"""GIN decoder (segment_sum aggregation + 2-layer MLP) on 8 trn2 NeuronCores.

Strategy (data-parallel over nodes):
  - Core c owns dst nodes [c*2500, (c+1)*2500), padded to 2560 columns.
  - The segment-sum becomes a dense matmul: h^T = x^T @ (Adj + I)_c where
    (Adj+I)_c[src, dst_local] = edge multiplicity (+1 on the diagonal for the
    GIN self-term).  Built on host from edge_index (pure index preprocessing),
    shipped as bf16.
  - Activations stay feature-major (transposed) through the MLP so weights act
    as the stationary (lhsT) operand: h1^T = W1 @ h^T + b1, out^T = W2 @ h1^T + b2.
  - Biases ride matmul_tile_kernel's accumulate_ap as host-broadcast tensors.
  - Output is produced transposed per core [8192, 2560]; host unpacks,
    crops and concatenates.

All device compute is bf16 matmul with f32 PSUM accumulation; output f32.
"""

import numpy as np
import ml_dtypes

P = 128
N_NODES = 20000
HIDDEN = 512
MIDDLE = 4352
VOCAB = 8192
NCORES = 8
ND = N_NODES // NCORES          # 2500 nodes per core
NDP = 2560                      # padded to 5*512
KSRC = 20096                    # 157*128, src contraction dim padded
BF16 = ml_dtypes.bfloat16

_BUILT = {}
LAST_RESULTS = None             # BassKernelResults of the last run (for test.py)


def _pack(a):
    """[K, M] row-major -> partition-tiled [P, K//P, M] (row r -> [r%P, r//P, :])."""
    K, M = a.shape
    assert K % P == 0, (K, M)
    return np.ascontiguousarray(a.reshape(K // P, P, M).transpose(1, 0, 2))


def _unpack(a):
    """[P, MB, N] -> [MB*P, N]."""
    Pp, MB, N = a.shape
    return np.ascontiguousarray(a.transpose(1, 0, 2)).reshape(MB * Pp, N)


def _build():
    if "nc" in _BUILT:
        return _BUILT["nc"]
    from concourse import bacc, mybir
    import concourse.tile as tile
    from concourse.kernels.tile_matmul import matmul_tile_kernel

    dt = mybir.dt
    nc = bacc.Bacc("TRN2", target_bir_lowering=False, debug=False,
                   num_devices=NCORES)

    x_kxm = nc.dram_tensor("x_kxm", [P, KSRC // P, HIDDEN], dt.bfloat16,
                           kind="ExternalInput").ap()
    adj_kxn = nc.dram_tensor("adj_kxn", [P, KSRC // P, NDP], dt.bfloat16,
                             kind="ExternalInput").ap()
    w1t_kxm = nc.dram_tensor("w1t_kxm", [P, HIDDEN // P, MIDDLE], dt.bfloat16,
                             kind="ExternalInput").ap()
    b1_mxn = nc.dram_tensor("b1_mxn", [P, MIDDLE // P, NDP], dt.bfloat16,
                            kind="ExternalInput").ap()
    w2t_kxm = nc.dram_tensor("w2t_kxm", [P, MIDDLE // P, VOCAB], dt.bfloat16,
                             kind="ExternalInput").ap()
    b2_mxn = nc.dram_tensor("b2_mxn", [P, VOCAB // P, NDP], dt.bfloat16,
                            kind="ExternalInput").ap()
    out_mxn = nc.dram_tensor("out_mxn", [P, VOCAB // P, NDP], dt.float32,
                             kind="ExternalOutput").ap()
    h_mxn = nc.dram_tensor("h_mxn", [P, HIDDEN // P, NDP], dt.bfloat16).ap()
    h1_mxn = nc.dram_tensor("h1_mxn", [P, MIDDLE // P, NDP], dt.bfloat16).ap()

    with tile.TileContext(nc) as tc:
        # h^T = x^T (Adj + I): K=20096 is 157 (prime) k-tiles, and M=512 means a
        # single m-tile, so caching kxn tiles would blow SBUF for zero reuse.
        matmul_tile_kernel(tc, x_kxm, adj_kxn, h_mxn, cache_tiles=False)
        # h1^T = W1 h^T + b1
        matmul_tile_kernel(tc, w1t_kxm, h_mxn, h1_mxn, accumulate_ap=b1_mxn)
        # out^T = W2 h1^T + b2
        matmul_tile_kernel(tc, w2t_kxm, h1_mxn, out_mxn, accumulate_ap=b2_mxn)
    nc.compile()
    _BUILT["nc"] = nc
    return nc


def _make_runner():
    """Build (once) a cached sharded-jit callable over the 8 cores.

    Returns dict with: fn(ins_dev, outs_prev) -> outs, names, avals, mesh,
    sharding.  Outputs are donated back in as the next call's (fully
    overwritten) output buffers, so steady-state calls move no host data.
    """
    if "runner" in _BUILT:
        return _BUILT["runner"]
    import jax
    from jax.experimental.shard_map import shard_map
    from jax.sharding import Mesh, NamedSharding, PartitionSpec
    from concourse import bass2jax, mybir

    nc = _build()
    bass2jax.install_neuronx_cc_hook()

    pid_name = (nc.partition_id_tensor.name
                if nc.partition_id_tensor is not None else None)
    in_names, out_names, out_avals = [], [], []
    for alloc in nc.m.functions[0].allocations:
        if not isinstance(alloc, mybir.MemoryLocationSet):
            continue
        name = alloc.memorylocations[0].name
        if alloc.kind == "ExternalInput":
            if name != pid_name:
                in_names.append(name)
        elif alloc.kind == "ExternalOutput":
            out_names.append(name)
            out_avals.append(jax.core.ShapedArray(
                tuple(alloc.tensor_shape), mybir.dt.np(alloc.dtype)))
    n_params = len(in_names)
    all_names = in_names + out_names
    if pid_name is not None:
        all_names = all_names + [pid_name]
    donate = tuple(range(n_params, n_params + len(out_names)))

    def _body(*args):
        operands = list(args)
        if pid_name is not None:
            operands.append(bass2jax.partition_id_tensor())
        outs = bass2jax._bass_exec_p.bind(
            *operands,
            out_avals=tuple(out_avals),
            in_names=tuple(all_names),
            out_names=tuple(out_names),
            lowering_input_output_aliases=(),
            sim_require_finite=True,
            sim_require_nnan=True,
            nc=nc,
        )
        return tuple(outs)

    devices = jax.devices()[:NCORES]
    mesh = Mesh(np.asarray(devices), ("core",))
    spec = PartitionSpec("core")
    in_specs = (spec,) * (n_params + len(out_names))
    out_specs = (spec,) * len(out_names)
    fn = jax.jit(
        shard_map(_body, mesh=mesh, in_specs=in_specs, out_specs=out_specs,
                  check_rep=False),
        donate_argnums=donate, keep_unused=True,
    )
    sharding = NamedSharding(mesh, spec)
    runner = dict(fn=fn, in_names=in_names, out_names=out_names,
                  out_avals=out_avals, sharding=sharding, mesh=mesh)
    _BUILT["runner"] = runner
    return runner


def _prep_device_inputs(in_maps):
    """device_put the concatenated per-core inputs; returns (ins_dev, zeros)."""
    import jax
    r = _make_runner()
    concat = [np.concatenate([m[name] for m in in_maps], axis=0)
              for name in r["in_names"]]
    ins_dev = [jax.device_put(a, r["sharding"]) for a in concat]
    zeros = [
        jax.jit(lambda a=av: jax.numpy.zeros(
            (NCORES * a.shape[0], *a.shape[1:]), a.dtype),
            out_shardings=r["sharding"])()
        for av in r["out_avals"]
    ]
    jax.block_until_ready(ins_dev + zeros)
    return ins_dev, zeros


def _run_once(ins_dev, out_bufs):
    import jax
    r = _make_runner()
    outs = r["fn"](*ins_dev, *out_bufs)
    jax.block_until_ready(outs)
    return outs


def kernel(x, edge_index, W1, b1, W2, b2):
    global LAST_RESULTS

    x = np.asarray(x, dtype=np.float32)
    edge_index = np.asarray(edge_index)
    W1 = np.asarray(W1, dtype=np.float32)
    b1 = np.asarray(b1, dtype=np.float32)
    W2 = np.asarray(W2, dtype=np.float32)
    b2 = np.asarray(b2, dtype=np.float32)

    src = edge_index[0].astype(np.int64)
    dst = edge_index[1].astype(np.int64)

    # --- host packing (index preprocessing + layout/dtype shuffles) ---
    x_pad = np.zeros((KSRC, HIDDEN), dtype=BF16)
    x_pad[:N_NODES] = x
    x_kxm = _pack(x_pad)

    # Adjacency with multiplicities + identity (GIN self term), per-core slabs.
    adj = np.zeros((KSRC, NCORES, NDP), dtype=np.int16)
    np.add.at(adj, (src, dst // ND, dst % ND), 1)
    d = np.arange(N_NODES)
    adj[d, d // ND, d % ND] += 1

    w1t_kxm = _pack(W1.T.astype(BF16))                      # [512, 4352]
    w2t_kxm = _pack(W2.T.astype(BF16))                      # [4352, 8192]
    b1_mxn = _pack(np.broadcast_to(b1.astype(BF16)[:, None], (MIDDLE, NDP)))
    b2_mxn = _pack(np.broadcast_to(b2.astype(BF16)[:, None], (VOCAB, NDP)))

    in_maps = []
    for c in range(NCORES):
        in_maps.append({
            "x_kxm": x_kxm,
            "adj_kxn": _pack(adj[:, c, :].astype(BF16)),
            "w1t_kxm": w1t_kxm,
            "b1_mxn": b1_mxn,
            "w2t_kxm": w2t_kxm,
            "b2_mxn": b2_mxn,
        })

    ins_dev, zeros = _prep_device_inputs(in_maps)
    outs = _run_once(ins_dev, zeros)
    LAST_RESULTS = dict(ins_dev=ins_dev, outs=outs)

    r = _make_runner()
    out_global = np.asarray(outs[r["out_names"].index("out_mxn")])
    out_global = out_global.reshape(NCORES, P, VOCAB // P, NDP)

    out = np.empty((N_NODES, VOCAB), dtype=np.float32)
    for c in range(NCORES):
        o = _unpack(out_global[c])                          # [8192, 2560]
        out[c * ND:(c + 1) * ND] = o[:, :ND].T
    return out


def bench(iters=5):
    """Steady-state per-call wall time (s); requires kernel() to have run."""
    import time
    import jax
    st = LAST_RESULTS
    assert st is not None, "run kernel() first"
    outs = st["outs"]
    times = []
    for _ in range(iters):
        t0 = time.perf_counter()
        outs = _run_once(st["ins_dev"], outs)
        times.append(time.perf_counter() - t0)
    st["outs"] = outs
    return times


# revision 5
# speedup vs baseline: 7.5689x; 7.5689x over previous
"""GIN decoder (segment_sum aggregation + 2-layer MLP) on 8 trn2 NeuronCores.

Strategy (data-parallel over nodes):
  - Core c owns dst nodes [c*2500, (c+1)*2500), padded to 2560 columns.
  - The segment-sum becomes a dense matmul: h^T = x^T @ (Adj + I)_c where
    (Adj+I)_c[src, dst_local] = edge multiplicity (+1 on the diagonal for the
    GIN self-term).  Built on host from edge_index (pure index preprocessing),
    shipped as bf16.
  - Activations stay feature-major (transposed) through the MLP so weights act
    as the stationary (lhsT) operand: h1^T = W1 @ h^T + b1, out^T = W2 @ h1^T + b2.
  - Biases ride matmul_tile_kernel's accumulate_ap as host-broadcast tensors.
  - Output is produced transposed per core [8192, 2560]; host unpacks,
    crops and concatenates.

All device compute is bf16 matmul with f32 PSUM accumulation; output f32.
"""

import numpy as np
import ml_dtypes

P = 128
N_NODES = 20000
HIDDEN = 512
MIDDLE = 4352
VOCAB = 8192
NCORES = 8
ND = N_NODES // NCORES          # 2500 nodes per core
NDP = 2560                      # padded to 5*512
KSRC = 20096                    # 157*128, src contraction dim padded
BF16 = ml_dtypes.bfloat16

_BUILT = {}
LAST_RESULTS = None             # BassKernelResults of the last run (for test.py)


def _pack(a):
    """[K, M] row-major -> partition-tiled [P, K//P, M] (row r -> [r%P, r//P, :])."""
    K, M = a.shape
    assert K % P == 0, (K, M)
    return np.ascontiguousarray(a.reshape(K // P, P, M).transpose(1, 0, 2))


def _unpack(a):
    """[P, MB, N] -> [MB*P, N]."""
    Pp, MB, N = a.shape
    return np.ascontiguousarray(a.transpose(1, 0, 2)).reshape(MB * Pp, N)


def _build():
    if "nc" in _BUILT:
        return _BUILT["nc"]
    from concourse import bacc, mybir
    import concourse.tile as tile
    from concourse.kernels.tile_matmul import matmul_tile_kernel

    dt = mybir.dt
    nc = bacc.Bacc("TRN2", target_bir_lowering=False, debug=False,
                   num_devices=NCORES)

    x_kxm = nc.dram_tensor("x_kxm", [P, KSRC // P, HIDDEN], dt.bfloat16,
                           kind="ExternalInput").ap()
    adj_kxn = nc.dram_tensor("adj_kxn", [P, KSRC // P, NDP], dt.bfloat16,
                             kind="ExternalInput").ap()
    w1t_kxm = nc.dram_tensor("w1t_kxm", [P, HIDDEN // P, MIDDLE], dt.bfloat16,
                             kind="ExternalInput").ap()
    b1_mxn = nc.dram_tensor("b1_mxn", [P, MIDDLE // P, NDP], dt.bfloat16,
                            kind="ExternalInput").ap()
    w2t_kxm = nc.dram_tensor("w2t_kxm", [P, MIDDLE // P, VOCAB], dt.bfloat16,
                             kind="ExternalInput").ap()
    b2_mxn = nc.dram_tensor("b2_mxn", [P, VOCAB // P, NDP], dt.bfloat16,
                            kind="ExternalInput").ap()
    out_mxn = nc.dram_tensor("out_mxn", [P, VOCAB // P, NDP], dt.float32,
                             kind="ExternalOutput").ap()
    h_mxn = nc.dram_tensor("h_mxn", [P, HIDDEN // P, NDP], dt.bfloat16).ap()
    h1_mxn = nc.dram_tensor("h1_mxn", [P, MIDDLE // P, NDP], dt.bfloat16).ap()

    with tile.TileContext(nc) as tc:
        # h^T = x^T (Adj + I): K=20096 is 157 (prime) k-tiles, and M=512 means a
        # single m-tile, so caching kxn tiles would blow SBUF for zero reuse.
        matmul_tile_kernel(tc, x_kxm, adj_kxn, h_mxn, cache_tiles=False)
        # h1^T = W1 h^T + b1
        matmul_tile_kernel(tc, w1t_kxm, h_mxn, h1_mxn, accumulate_ap=b1_mxn)
        # out^T = W2 h1^T + b2
        matmul_tile_kernel(tc, w2t_kxm, h1_mxn, out_mxn, accumulate_ap=b2_mxn)
    nc.compile()
    _BUILT["nc"] = nc
    return nc


def _make_runner():
    """Build (once) a cached sharded-jit callable over the 8 cores.

    Returns dict with: fn(ins_dev, outs_prev) -> outs, names, avals, mesh,
    sharding.  Outputs are donated back in as the next call's (fully
    overwritten) output buffers, so steady-state calls move no host data.
    """
    if "runner" in _BUILT:
        return _BUILT["runner"]
    import jax
    from jax.experimental.shard_map import shard_map
    from jax.sharding import Mesh, NamedSharding, PartitionSpec
    from concourse import bass2jax, mybir

    nc = _build()
    bass2jax.install_neuronx_cc_hook()

    pid_name = (nc.partition_id_tensor.name
                if nc.partition_id_tensor is not None else None)
    in_names, out_names, out_avals = [], [], []
    for alloc in nc.m.functions[0].allocations:
        if not isinstance(alloc, mybir.MemoryLocationSet):
            continue
        name = alloc.memorylocations[0].name
        if alloc.kind == "ExternalInput":
            if name != pid_name:
                in_names.append(name)
        elif alloc.kind == "ExternalOutput":
            out_names.append(name)
            out_avals.append(jax.core.ShapedArray(
                tuple(alloc.tensor_shape), mybir.dt.np(alloc.dtype)))
    n_params = len(in_names)
    all_names = in_names + out_names
    if pid_name is not None:
        all_names = all_names + [pid_name]
    donate = tuple(range(n_params, n_params + len(out_names)))

    def _body(*args):
        operands = list(args)
        if pid_name is not None:
            operands.append(bass2jax.partition_id_tensor())
        outs = bass2jax._bass_exec_p.bind(
            *operands,
            out_avals=tuple(out_avals),
            in_names=tuple(all_names),
            out_names=tuple(out_names),
            lowering_input_output_aliases=(),
            sim_require_finite=True,
            sim_require_nnan=True,
            nc=nc,
        )
        return tuple(outs)

    devices = jax.devices()[:NCORES]
    mesh = Mesh(np.asarray(devices), ("core",))
    spec = PartitionSpec("core")
    in_specs = (spec,) * (n_params + len(out_names))
    out_specs = (spec,) * len(out_names)
    fn = jax.jit(
        shard_map(_body, mesh=mesh, in_specs=in_specs, out_specs=out_specs,
                  check_rep=False),
        donate_argnums=donate, keep_unused=True,
    )
    sharding = NamedSharding(mesh, spec)
    runner = dict(fn=fn, in_names=in_names, out_names=out_names,
                  out_avals=out_avals, sharding=sharding, mesh=mesh)
    _BUILT["runner"] = runner
    return runner


def _prep_device_inputs(in_maps):
    """device_put the concatenated per-core inputs; returns (ins_dev, zeros)."""
    import jax
    r = _make_runner()
    concat = [np.concatenate([m[name] for m in in_maps], axis=0)
              for name in r["in_names"]]
    ins_dev = [jax.device_put(a, r["sharding"]) for a in concat]
    zeros = [
        jax.jit(lambda a=av: jax.numpy.zeros(
            (NCORES * a.shape[0], *a.shape[1:]), a.dtype),
            out_shardings=r["sharding"])()
        for av in r["out_avals"]
    ]
    jax.block_until_ready(ins_dev + zeros)
    return ins_dev, zeros


def _run_once(ins_dev, out_bufs):
    import jax
    r = _make_runner()
    outs = r["fn"](*ins_dev, *out_bufs)
    jax.block_until_ready(outs)
    return outs


def kernel(x, edge_index, W1, b1, W2, b2):
    global LAST_RESULTS

    x = np.asarray(x, dtype=np.float32)
    edge_index = np.asarray(edge_index)
    W1 = np.asarray(W1, dtype=np.float32)
    b1 = np.asarray(b1, dtype=np.float32)
    W2 = np.asarray(W2, dtype=np.float32)
    b2 = np.asarray(b2, dtype=np.float32)

    src = edge_index[0].astype(np.int64)
    dst = edge_index[1].astype(np.int64)

    # --- host packing (index preprocessing + layout/dtype shuffles) ---
    x_pad = np.zeros((KSRC, HIDDEN), dtype=BF16)
    x_pad[:N_NODES] = x
    x_kxm = _pack(x_pad)

    # Adjacency with multiplicities + identity (GIN self term), per-core slabs.
    adj = np.zeros((KSRC, NCORES, NDP), dtype=np.int16)
    np.add.at(adj, (src, dst // ND, dst % ND), 1)
    d = np.arange(N_NODES)
    adj[d, d // ND, d % ND] += 1

    w1t_kxm = _pack(W1.T.astype(BF16))                      # [512, 4352]
    w2t_kxm = _pack(W2.T.astype(BF16))                      # [4352, 8192]
    b1_mxn = _pack(np.broadcast_to(b1.astype(BF16)[:, None], (MIDDLE, NDP)))
    b2_mxn = _pack(np.broadcast_to(b2.astype(BF16)[:, None], (VOCAB, NDP)))

    in_maps = []
    for c in range(NCORES):
        in_maps.append({
            "x_kxm": x_kxm,
            "adj_kxn": _pack(adj[:, c, :].astype(BF16)),
            "w1t_kxm": w1t_kxm,
            "b1_mxn": b1_mxn,
            "w2t_kxm": w2t_kxm,
            "b2_mxn": b2_mxn,
        })

    ins_dev, zeros = _prep_device_inputs(in_maps)
    outs = _run_once(ins_dev, zeros)
    LAST_RESULTS = dict(ins_dev=ins_dev, outs=outs)

    r = _make_runner()
    out_global = np.asarray(outs[r["out_names"].index("out_mxn")])
    out_global = out_global.reshape(NCORES, P, VOCAB // P, NDP)

    out = np.empty((N_NODES, VOCAB), dtype=np.float32)
    for c in range(NCORES):
        o = _unpack(out_global[c])                          # [8192, 2560]
        out[c * ND:(c + 1) * ND] = o[:, :ND].T
    return out


def bench(iters=5):
    """Steady-state per-call wall time (s); requires kernel() to have run."""
    import time
    import jax
    st = LAST_RESULTS
    assert st is not None, "run kernel() first"
    outs = st["outs"]
    times = []
    for _ in range(iters):
        t0 = time.perf_counter()
        outs = _run_once(st["ins_dev"], outs)
        times.append(time.perf_counter() - t0)
    st["outs"] = outs
    return times


def bench_pipelined(iters=8):
    """Dispatch `iters` chained calls without blocking, block once.

    Successive calls are serialized on-device by the donated-output data
    dependence, while host dispatch overlaps — the per-iter slope is the
    device execution time.
    """
    import time
    import jax
    r = _make_runner()
    st = LAST_RESULTS
    assert st is not None, "run kernel() first"
    outs = st["outs"]
    # warm: one blocked call so everything is resident
    outs = _run_once(st["ins_dev"], outs)
    t0 = time.perf_counter()
    outs = r["fn"](*st["ins_dev"], *outs)
    jax.block_until_ready(outs)
    t1 = time.perf_counter() - t0
    t0 = time.perf_counter()
    for _ in range(iters):
        outs = r["fn"](*st["ins_dev"], *outs)
    jax.block_until_ready(outs)
    tN = time.perf_counter() - t0
    st["outs"] = outs
    per_iter = (tN - t1) / (iters - 1)
    return dict(t1=t1, tN=tN, iters=iters, per_iter=per_iter)


# revision 13
# speedup vs baseline: 9.8574x; 1.3024x over previous
"""GIN decoder (segment_sum aggregation + 2-layer MLP) on 8 trn2 NeuronCores.

Strategy (data-parallel over nodes):
  - Core c owns dst nodes [c*2500, (c+1)*2500), padded to 2560 columns.
  - The segment-sum becomes a dense matmul: h^T = x^T @ (Adj + I)_c where
    (Adj+I)_c[src, dst_local] = edge multiplicity (+1 on the diagonal for the
    GIN self-term).  Built on host from edge_index (pure index preprocessing),
    shipped as bf16.
  - Activations stay feature-major (transposed) through the MLP so weights act
    as the stationary (lhsT) operand: h1^T = W1 @ h^T + b1, out^T = W2 @ h1^T + b2.
  - Biases ride matmul_tile_kernel's accumulate_ap as host-broadcast tensors.
  - Output is produced transposed per core [8192, 2560]; host unpacks,
    crops and concatenates.

All device compute is bf16 matmul with f32 PSUM accumulation; output f32.
"""

import numpy as np
import ml_dtypes

P = 128
N_NODES = 20000
HIDDEN = 512
MIDDLE = 4352
VOCAB = 8192
NCORES = 8
ND = N_NODES // NCORES          # 2500 nodes per core
NDP = 2560                      # padded to 5*512
KSRC = 20096                    # 157*128, src contraction dim padded
BF16 = ml_dtypes.bfloat16

NBLK = NDP // P                 # 20 dst blocks of 128 per core
T_TILES = 36                    # 128-edge tiles per dst block (4608 cap/block)
ZERO_ROW = N_NODES              # gather target row holding zeros

_BUILT = {}
LAST_RESULTS = None             # state of the last run (for test.py)


def _pack(a):
    """[K, M] row-major -> partition-tiled [P, K//P, M] (row r -> [r%P, r//P, :])."""
    K, M = a.shape
    assert K % P == 0, (K, M)
    return np.ascontiguousarray(a.reshape(K // P, P, M).transpose(1, 0, 2))


def _unpack(a):
    """[P, MB, N] -> [MB*P, N]."""
    Pp, MB, N = a.shape
    return np.ascontiguousarray(a.transpose(1, 0, 2)).reshape(MB * Pp, N)


def _agg_stage(ctx, tc, x_rows, src_ids, dst_ids, colidx, h_mxn, T):
    """h^T = gathered segment-sum, written feature-major to h_mxn.

    Edges (incl. self-loops for the GIN x-term) are host-sorted into
    (dst-block, tile) buckets of 128; each 128-edge tile gathers x rows by
    src id (indirect DMA) and scatter-adds within the 128-wide dst block via
    a one-hot matmul accumulated in PSUM; the block result is PE-transposed
    to feature-major.
    """
    import concourse.bass as bass
    from concourse import mybir
    from concourse.masks import make_identity

    nc = tc.nc
    dt = mybir.dt
    sb = ctx.enter_context(tc.tile_pool(name="agg_sb", bufs=8))
    idp = ctx.enter_context(tc.tile_pool(name="agg_idx", bufs=3))
    psp = ctx.enter_context(tc.tile_pool(name="agg_ps", bufs=2, space="PSUM"))
    tpp = ctx.enter_context(tc.tile_pool(name="agg_tp", bufs=2, space="PSUM"))
    const = ctx.enter_context(tc.tile_pool(name="agg_const", bufs=1))

    colidx_sb = const.tile([P, P], dt.float32)
    nc.sync.dma_start(colidx_sb[:], colidx[:])
    ident = const.tile([P, P], dt.float32)
    make_identity(nc, ident[:])

    for b in range(NBLK):
        sid = idp.tile([P, T], dt.int32)
        nc.sync.dma_start(sid[:], src_ids[b])
        did = idp.tile([P, T], dt.float32)
        nc.sync.dma_start(did[:], dst_ids[b])

        ps = psp.tile([P, HIDDEN], dt.float32, space="PSUM")
        for t in range(T):
            g = sb.tile([P, HIDDEN], dt.bfloat16, name="gather")
            nc.gpsimd.indirect_dma_start(
                out=g[:], out_offset=None, in_=x_rows[:],
                in_offset=bass.IndirectOffsetOnAxis(ap=sid[:, t:t + 1], axis=0))
            oh = sb.tile([P, P], dt.bfloat16, name="onehot")
            nc.vector.tensor_tensor(
                out=oh[:], in0=did[:, t:t + 1].to_broadcast([P, P]),
                in1=colidx_sb[:], op=mybir.AluOpType.is_equal)
            nc.tensor.matmul(ps[:], lhsT=oh[:], rhs=g[:],
                             start=(t == 0), stop=(t == T - 1))

        hsb = sb.tile([P, HIDDEN], dt.float32, name="hsb")
        nc.scalar.copy(hsb[:], ps[:])
        for j in range(HIDDEN // P):
            tp = tpp.tile([P, P], dt.float32, space="PSUM")
            nc.tensor.transpose(out=tp[:], in_=hsb[:, j * P:(j + 1) * P],
                                identity=ident[:])
            htp = sb.tile([P, P], dt.bfloat16, name="htp")
            nc.vector.tensor_copy(htp[:], tp[:])
            nc.sync.dma_start(h_mxn[:, j, b * P:(b + 1) * P], htp[:])


def _build(T=T_TILES):
    key = ("nc", T)
    if key in _BUILT:
        return _BUILT[key]
    from contextlib import ExitStack
    from concourse import bacc, mybir
    import concourse.tile as tile
    from concourse.kernels.tile_matmul import matmul_tile_kernel

    dt = mybir.dt
    nc = bacc.Bacc("TRN2", target_bir_lowering=False, debug=False,
                   num_devices=NCORES)

    x_rows = nc.dram_tensor("x_rows", [KSRC, HIDDEN], dt.bfloat16,
                            kind="ExternalInput").ap()
    src_ids = nc.dram_tensor("src_ids", [NBLK, P, T], dt.int32,
                             kind="ExternalInput").ap()
    dst_ids = nc.dram_tensor("dst_ids", [NBLK, P, T], dt.float32,
                             kind="ExternalInput").ap()
    colidx = nc.dram_tensor("colidx", [P, P], dt.float32,
                            kind="ExternalInput").ap()
    w1t_kxm = nc.dram_tensor("w1t_kxm", [P, HIDDEN // P, MIDDLE], dt.bfloat16,
                             kind="ExternalInput").ap()
    b1_mxn = nc.dram_tensor("b1_mxn", [P, MIDDLE // P, NDP], dt.bfloat16,
                            kind="ExternalInput").ap()
    w2t_kxm = nc.dram_tensor("w2t_kxm", [P, MIDDLE // P, VOCAB], dt.bfloat16,
                             kind="ExternalInput").ap()
    b2_mxn = nc.dram_tensor("b2_mxn", [P, VOCAB // P, NDP], dt.bfloat16,
                            kind="ExternalInput").ap()
    out_mxn = nc.dram_tensor("out_mxn", [P, VOCAB // P, NDP], dt.float32,
                             kind="ExternalOutput").ap()
    h_mxn = nc.dram_tensor("h_mxn", [P, HIDDEN // P, NDP], dt.bfloat16).ap()
    h1_mxn = nc.dram_tensor("h1_mxn", [P, MIDDLE // P, NDP], dt.bfloat16).ap()

    with tile.TileContext(nc) as tc:
        with ExitStack() as ctx:
            _agg_stage(ctx, tc, x_rows, src_ids, dst_ids, colidx, h_mxn, T)
            # h1^T = W1 h^T + b1
            matmul_tile_kernel(tc, w1t_kxm, h_mxn, h1_mxn, accumulate_ap=b1_mxn)
            # out^T = W2 h1^T + b2
            matmul_tile_kernel(tc, w2t_kxm, h1_mxn, out_mxn, accumulate_ap=b2_mxn)
    nc.compile()
    _BUILT[key] = nc
    return nc


def _make_runner(T=T_TILES):
    """Build (once) a cached sharded-jit callable over the 8 cores.

    Returns dict with: fn(ins_dev, outs_prev) -> outs, names, avals, mesh,
    sharding.  Outputs are donated back in as the next call's (fully
    overwritten) output buffers, so steady-state calls move no host data.
    """
    rkey = ("runner", T)
    if rkey in _BUILT:
        return _BUILT[rkey]
    import jax
    from jax.experimental.shard_map import shard_map
    from jax.sharding import Mesh, NamedSharding, PartitionSpec
    from concourse import bass2jax, mybir

    nc = _build(T)
    bass2jax.install_neuronx_cc_hook()

    pid_name = (nc.partition_id_tensor.name
                if nc.partition_id_tensor is not None else None)
    in_names, out_names, out_avals = [], [], []
    for alloc in nc.m.functions[0].allocations:
        if not isinstance(alloc, mybir.MemoryLocationSet):
            continue
        name = alloc.memorylocations[0].name
        if alloc.kind == "ExternalInput":
            if name != pid_name:
                in_names.append(name)
        elif alloc.kind == "ExternalOutput":
            out_names.append(name)
            out_avals.append(jax.core.ShapedArray(
                tuple(alloc.tensor_shape), mybir.dt.np(alloc.dtype)))
    n_params = len(in_names)
    all_names = in_names + out_names
    if pid_name is not None:
        all_names = all_names + [pid_name]
    donate = tuple(range(n_params, n_params + len(out_names)))

    def _body(*args):
        operands = list(args)
        if pid_name is not None:
            operands.append(bass2jax.partition_id_tensor())
        outs = bass2jax._bass_exec_p.bind(
            *operands,
            out_avals=tuple(out_avals),
            in_names=tuple(all_names),
            out_names=tuple(out_names),
            lowering_input_output_aliases=(),
            sim_require_finite=True,
            sim_require_nnan=True,
            nc=nc,
        )
        return tuple(outs)

    devices = jax.devices()[:NCORES]
    mesh = Mesh(np.asarray(devices), ("core",))
    spec = PartitionSpec("core")
    in_specs = (spec,) * (n_params + len(out_names))
    out_specs = (spec,) * len(out_names)
    fn = jax.jit(
        shard_map(_body, mesh=mesh, in_specs=in_specs, out_specs=out_specs,
                  check_rep=False),
        donate_argnums=donate, keep_unused=True,
    )
    sharding = NamedSharding(mesh, spec)
    runner = dict(fn=fn, in_names=in_names, out_names=out_names,
                  out_avals=out_avals, sharding=sharding, mesh=mesh)
    _BUILT[rkey] = runner
    return runner


def _prep_device_inputs(in_maps, T=T_TILES):
    """device_put the concatenated per-core inputs; returns (ins_dev, zeros)."""
    import jax
    r = _make_runner(T)
    concat = [np.concatenate([m[name] for m in in_maps], axis=0)
              for name in r["in_names"]]
    ins_dev = [jax.device_put(a, r["sharding"]) for a in concat]
    zeros = [
        jax.jit(lambda a=av: jax.numpy.zeros(
            (NCORES * a.shape[0], *a.shape[1:]), a.dtype),
            out_shardings=r["sharding"])()
        for av in r["out_avals"]
    ]
    jax.block_until_ready(ins_dev + zeros)
    return ins_dev, zeros


def _run_once(ins_dev, out_bufs, T=T_TILES):
    import jax
    r = _make_runner(T)
    outs = r["fn"](*ins_dev, *out_bufs)
    jax.block_until_ready(outs)
    return outs


def kernel(x, edge_index, W1, b1, W2, b2):
    global LAST_RESULTS

    x = np.asarray(x, dtype=np.float32)
    edge_index = np.asarray(edge_index)
    W1 = np.asarray(W1, dtype=np.float32)
    b1 = np.asarray(b1, dtype=np.float32)
    W2 = np.asarray(W2, dtype=np.float32)
    b2 = np.asarray(b2, dtype=np.float32)

    src = edge_index[0].astype(np.int64)
    dst = edge_index[1].astype(np.int64)

    # --- host packing (index preprocessing + layout/dtype shuffles) ---
    x_rows = np.zeros((KSRC, HIDDEN), dtype=BF16)
    x_rows[:N_NODES] = x

    # Edge list incl. self-loops (the GIN (1+eps)*x_i term, eps=0), bucketed
    # by (core, dst-block of 128) and padded to T*128 per bucket with edges
    # from the all-zeros row.
    allsrc = np.concatenate([src, np.arange(N_NODES, dtype=np.int64)])
    alldst = np.concatenate([dst, np.arange(N_NODES, dtype=np.int64)])
    core = alldst // ND
    local = alldst % ND
    blk = local // P
    within = (local % P).astype(np.int32)
    bucket = core * NBLK + blk
    order = np.argsort(bucket, kind="stable")
    bs = bucket[order]
    counts = np.bincount(bucket, minlength=NCORES * NBLK)
    T = T_TILES
    maxc = int(counts.max())
    if maxc > T * P:
        T = -(-maxc // P)       # fallback: recompile with a bigger T
    cap = T * P
    starts = np.zeros(NCORES * NBLK, dtype=np.int64)
    np.cumsum(counts[:-1], out=starts[1:])
    pos = np.arange(bs.size, dtype=np.int64) - starts[bs]
    src_pad = np.full((NCORES * NBLK, cap), ZERO_ROW, dtype=np.int32)
    dst_pad = np.zeros((NCORES * NBLK, cap), dtype=np.float32)
    src_pad[bs, pos] = allsrc[order].astype(np.int32)
    dst_pad[bs, pos] = within[order]
    # [ncores*nblk, T*P] -> [ncores, nblk, P, T] (tile-major -> [P, T] slabs)
    src_pad = src_pad.reshape(NCORES, NBLK, T, P).transpose(0, 1, 3, 2)
    dst_pad = dst_pad.reshape(NCORES, NBLK, T, P).transpose(0, 1, 3, 2)

    colidx = np.broadcast_to(np.arange(P, dtype=np.float32)[None, :], (P, P))
    colidx = np.ascontiguousarray(colidx)
    w1t_kxm = _pack(W1.T.astype(BF16))                      # [512, 4352]
    w2t_kxm = _pack(W2.T.astype(BF16))                      # [4352, 8192]
    b1_mxn = _pack(np.broadcast_to(b1.astype(BF16)[:, None], (MIDDLE, NDP)))
    b2_mxn = _pack(np.broadcast_to(b2.astype(BF16)[:, None], (VOCAB, NDP)))

    in_maps = []
    for c in range(NCORES):
        in_maps.append({
            "x_rows": x_rows,
            "src_ids": np.ascontiguousarray(src_pad[c]),
            "dst_ids": np.ascontiguousarray(dst_pad[c]),
            "colidx": colidx,
            "w1t_kxm": w1t_kxm,
            "b1_mxn": b1_mxn,
            "w2t_kxm": w2t_kxm,
            "b2_mxn": b2_mxn,
        })

    ins_dev, zeros = _prep_device_inputs(in_maps, T)
    outs = _run_once(ins_dev, zeros, T)
    LAST_RESULTS = dict(ins_dev=ins_dev, outs=outs, T=T)

    r = _make_runner(T)
    out_global = np.asarray(outs[r["out_names"].index("out_mxn")])
    out_global = out_global.reshape(NCORES, P, VOCAB // P, NDP)

    out = np.empty((N_NODES, VOCAB), dtype=np.float32)
    for c in range(NCORES):
        o = _unpack(out_global[c])                          # [8192, 2560]
        out[c * ND:(c + 1) * ND] = o[:, :ND].T
    return out


def bench(iters=5):
    """Steady-state per-call wall time (s); requires kernel() to have run."""
    import time
    import jax
    st = LAST_RESULTS
    assert st is not None, "run kernel() first"
    outs = st["outs"]
    times = []
    for _ in range(iters):
        t0 = time.perf_counter()
        outs = _run_once(st["ins_dev"], outs, st["T"])
        times.append(time.perf_counter() - t0)
    st["outs"] = outs
    return times


def bench_pipelined(iters=8):
    """Dispatch `iters` chained calls without blocking, block once.

    Successive calls are serialized on-device by the donated-output data
    dependence, while host dispatch overlaps — the per-iter slope is the
    device execution time.
    """
    import time
    import jax
    st = LAST_RESULTS
    assert st is not None, "run kernel() first"
    r = _make_runner(st["T"])
    outs = st["outs"]
    # warm: one blocked call so everything is resident
    outs = _run_once(st["ins_dev"], outs, st["T"])
    t0 = time.perf_counter()
    outs = r["fn"](*st["ins_dev"], *outs)
    jax.block_until_ready(outs)
    t1 = time.perf_counter() - t0
    t0 = time.perf_counter()
    for _ in range(iters):
        outs = r["fn"](*st["ins_dev"], *outs)
    jax.block_until_ready(outs)
    tN = time.perf_counter() - t0
    st["outs"] = outs
    per_iter = (tN - t1) / (iters - 1)
    return dict(t1=t1, tN=tN, iters=iters, per_iter=per_iter)


# revision 14
# speedup vs baseline: 9.9674x; 1.0112x over previous
"""GIN decoder (segment_sum aggregation + 2-layer MLP) on 8 trn2 NeuronCores.

Strategy (data-parallel over dst nodes; ~4.5 ms/core steady state):
  - Core c owns dst nodes [c*2500, (c+1)*2500), padded to 2560 columns.
  - Aggregation (segment-sum incl. the GIN self-term as explicit self-loops):
    edges are host-bucketed by (core, 128-wide dst block) and padded to
    T*128 per bucket with edges from an all-zeros x row.  On device, each
    128-edge tile gathers x rows by src id (indirect DMA) and scatter-adds
    into its dst block via a one-hot [128e x 128dst] matmul accumulated in
    PSUM; block results are PE-transposed to feature-major h^T.
  - Activations stay feature-major through the MLP so weights act as the
    stationary (lhsT) operand: h1^T = W1 h^T + b1, out^T = W2 h1^T + b2 via
    matmul_tile_kernel; biases ride accumulate_ap as host-broadcast tensors.
  - Output is produced transposed per core [8192, 2560]; host unpacks,
    crops and concatenates.

All device compute is bf16 matmul with f32 PSUM accumulation; output f32.
The MLP matmuls are at the bf16 tensor-engine roofline (~4.1 ms/core);
aggregation adds ~0.3 ms of PE time with gather DMA hidden underneath.
"""

import numpy as np
import ml_dtypes

P = 128
N_NODES = 20000
HIDDEN = 512
MIDDLE = 4352
VOCAB = 8192
NCORES = 8
ND = N_NODES // NCORES          # 2500 nodes per core
NDP = 2560                      # padded to 5*512
KSRC = 20096                    # 157*128, src contraction dim padded
BF16 = ml_dtypes.bfloat16

NBLK = NDP // P                 # 20 dst blocks of 128 per core
T_TILES = 36                    # 128-edge tiles per dst block (4608 cap/block)
ZERO_ROW = N_NODES              # gather target row holding zeros

_BUILT = {}
LAST_RESULTS = None             # state of the last run (for test.py)


def _pack(a):
    """[K, M] row-major -> partition-tiled [P, K//P, M] (row r -> [r%P, r//P, :])."""
    K, M = a.shape
    assert K % P == 0, (K, M)
    return np.ascontiguousarray(a.reshape(K // P, P, M).transpose(1, 0, 2))


def _unpack(a):
    """[P, MB, N] -> [MB*P, N]."""
    Pp, MB, N = a.shape
    return np.ascontiguousarray(a.transpose(1, 0, 2)).reshape(MB * Pp, N)


def _agg_stage(ctx, tc, x_rows, src_ids, dst_ids, colidx, h_mxn, T):
    """h^T = gathered segment-sum, written feature-major to h_mxn.

    Edges (incl. self-loops for the GIN x-term) are host-sorted into
    (dst-block, tile) buckets of 128; each 128-edge tile gathers x rows by
    src id (indirect DMA) and scatter-adds within the 128-wide dst block via
    a one-hot matmul accumulated in PSUM; the block result is PE-transposed
    to feature-major.
    """
    import concourse.bass as bass
    from concourse import mybir
    from concourse.masks import make_identity

    nc = tc.nc
    dt = mybir.dt
    sb = ctx.enter_context(tc.tile_pool(name="agg_sb", bufs=8))
    idp = ctx.enter_context(tc.tile_pool(name="agg_idx", bufs=3))
    psp = ctx.enter_context(tc.tile_pool(name="agg_ps", bufs=2, space="PSUM"))
    tpp = ctx.enter_context(tc.tile_pool(name="agg_tp", bufs=2, space="PSUM"))
    const = ctx.enter_context(tc.tile_pool(name="agg_const", bufs=1))

    colidx_sb = const.tile([P, P], dt.float32)
    nc.sync.dma_start(colidx_sb[:], colidx[:])
    ident = const.tile([P, P], dt.float32)
    make_identity(nc, ident[:])

    for b in range(NBLK):
        sid = idp.tile([P, T], dt.int32)
        nc.sync.dma_start(sid[:], src_ids[b])
        did = idp.tile([P, T], dt.float32)
        nc.sync.dma_start(did[:], dst_ids[b])

        ps = psp.tile([P, HIDDEN], dt.float32, space="PSUM")
        for t in range(T):
            g = sb.tile([P, HIDDEN], dt.bfloat16, name="gather")
            nc.gpsimd.indirect_dma_start(
                out=g[:], out_offset=None, in_=x_rows[:],
                in_offset=bass.IndirectOffsetOnAxis(ap=sid[:, t:t + 1], axis=0))
            oh = sb.tile([P, P], dt.bfloat16, name="onehot")
            nc.vector.tensor_tensor(
                out=oh[:], in0=did[:, t:t + 1].to_broadcast([P, P]),
                in1=colidx_sb[:], op=mybir.AluOpType.is_equal)
            nc.tensor.matmul(ps[:], lhsT=oh[:], rhs=g[:],
                             start=(t == 0), stop=(t == T - 1))

        hsb = sb.tile([P, HIDDEN], dt.float32, name="hsb")
        nc.scalar.copy(hsb[:], ps[:])
        for j in range(HIDDEN // P):
            tp = tpp.tile([P, P], dt.float32, space="PSUM")
            nc.tensor.transpose(out=tp[:], in_=hsb[:, j * P:(j + 1) * P],
                                identity=ident[:])
            htp = sb.tile([P, P], dt.bfloat16, name="htp")
            nc.vector.tensor_copy(htp[:], tp[:])
            nc.sync.dma_start(h_mxn[:, j, b * P:(b + 1) * P], htp[:])


def _build(T=T_TILES):
    key = ("nc", T)
    if key in _BUILT:
        return _BUILT[key]
    from contextlib import ExitStack
    from concourse import bacc, mybir
    import concourse.tile as tile
    from concourse.kernels.tile_matmul import matmul_tile_kernel

    dt = mybir.dt
    nc = bacc.Bacc("TRN2", target_bir_lowering=False, debug=False,
                   num_devices=NCORES)

    x_rows = nc.dram_tensor("x_rows", [KSRC, HIDDEN], dt.bfloat16,
                            kind="ExternalInput").ap()
    src_ids = nc.dram_tensor("src_ids", [NBLK, P, T], dt.int32,
                             kind="ExternalInput").ap()
    dst_ids = nc.dram_tensor("dst_ids", [NBLK, P, T], dt.float32,
                             kind="ExternalInput").ap()
    colidx = nc.dram_tensor("colidx", [P, P], dt.float32,
                            kind="ExternalInput").ap()
    w1t_kxm = nc.dram_tensor("w1t_kxm", [P, HIDDEN // P, MIDDLE], dt.bfloat16,
                             kind="ExternalInput").ap()
    b1_mxn = nc.dram_tensor("b1_mxn", [P, MIDDLE // P, NDP], dt.bfloat16,
                            kind="ExternalInput").ap()
    w2t_kxm = nc.dram_tensor("w2t_kxm", [P, MIDDLE // P, VOCAB], dt.bfloat16,
                             kind="ExternalInput").ap()
    b2_mxn = nc.dram_tensor("b2_mxn", [P, VOCAB // P, NDP], dt.bfloat16,
                            kind="ExternalInput").ap()
    out_mxn = nc.dram_tensor("out_mxn", [P, VOCAB // P, NDP], dt.float32,
                             kind="ExternalOutput").ap()
    h_mxn = nc.dram_tensor("h_mxn", [P, HIDDEN // P, NDP], dt.bfloat16).ap()
    h1_mxn = nc.dram_tensor("h1_mxn", [P, MIDDLE // P, NDP], dt.bfloat16).ap()

    with tile.TileContext(nc) as tc:
        with ExitStack() as ctx:
            _agg_stage(ctx, tc, x_rows, src_ids, dst_ids, colidx, h_mxn, T)
            # h1^T = W1 h^T + b1
            matmul_tile_kernel(tc, w1t_kxm, h_mxn, h1_mxn, accumulate_ap=b1_mxn)
            # out^T = W2 h1^T + b2
            matmul_tile_kernel(tc, w2t_kxm, h1_mxn, out_mxn, accumulate_ap=b2_mxn)
    nc.compile()
    _BUILT[key] = nc
    return nc


def _make_runner(T=T_TILES):
    """Build (once) a cached sharded-jit callable over the 8 cores.

    Returns dict with: fn(ins_dev, outs_prev) -> outs, names, avals, mesh,
    sharding.  Outputs are donated back in as the next call's (fully
    overwritten) output buffers, so steady-state calls move no host data.
    """
    rkey = ("runner", T)
    if rkey in _BUILT:
        return _BUILT[rkey]
    import jax
    from jax.experimental.shard_map import shard_map
    from jax.sharding import Mesh, NamedSharding, PartitionSpec
    from concourse import bass2jax, mybir

    nc = _build(T)
    bass2jax.install_neuronx_cc_hook()

    pid_name = (nc.partition_id_tensor.name
                if nc.partition_id_tensor is not None else None)
    in_names, out_names, out_avals = [], [], []
    for alloc in nc.m.functions[0].allocations:
        if not isinstance(alloc, mybir.MemoryLocationSet):
            continue
        name = alloc.memorylocations[0].name
        if alloc.kind == "ExternalInput":
            if name != pid_name:
                in_names.append(name)
        elif alloc.kind == "ExternalOutput":
            out_names.append(name)
            out_avals.append(jax.core.ShapedArray(
                tuple(alloc.tensor_shape), mybir.dt.np(alloc.dtype)))
    n_params = len(in_names)
    all_names = in_names + out_names
    if pid_name is not None:
        all_names = all_names + [pid_name]
    donate = tuple(range(n_params, n_params + len(out_names)))

    def _body(*args):
        operands = list(args)
        if pid_name is not None:
            operands.append(bass2jax.partition_id_tensor())
        outs = bass2jax._bass_exec_p.bind(
            *operands,
            out_avals=tuple(out_avals),
            in_names=tuple(all_names),
            out_names=tuple(out_names),
            lowering_input_output_aliases=(),
            sim_require_finite=True,
            sim_require_nnan=True,
            nc=nc,
        )
        return tuple(outs)

    devices = jax.devices()[:NCORES]
    mesh = Mesh(np.asarray(devices), ("core",))
    spec = PartitionSpec("core")
    in_specs = (spec,) * (n_params + len(out_names))
    out_specs = (spec,) * len(out_names)
    fn = jax.jit(
        shard_map(_body, mesh=mesh, in_specs=in_specs, out_specs=out_specs,
                  check_rep=False),
        donate_argnums=donate, keep_unused=True,
    )
    sharding = NamedSharding(mesh, spec)
    runner = dict(fn=fn, in_names=in_names, out_names=out_names,
                  out_avals=out_avals, sharding=sharding, mesh=mesh)
    _BUILT[rkey] = runner
    return runner


def _prep_device_inputs(in_maps, T=T_TILES):
    """device_put the concatenated per-core inputs; returns (ins_dev, zeros)."""
    import jax
    r = _make_runner(T)
    concat = [np.concatenate([m[name] for m in in_maps], axis=0)
              for name in r["in_names"]]
    ins_dev = [jax.device_put(a, r["sharding"]) for a in concat]
    zeros = [
        jax.jit(lambda a=av: jax.numpy.zeros(
            (NCORES * a.shape[0], *a.shape[1:]), a.dtype),
            out_shardings=r["sharding"])()
        for av in r["out_avals"]
    ]
    jax.block_until_ready(ins_dev + zeros)
    return ins_dev, zeros


def _run_once(ins_dev, out_bufs, T=T_TILES):
    import jax
    r = _make_runner(T)
    outs = r["fn"](*ins_dev, *out_bufs)
    jax.block_until_ready(outs)
    return outs


def kernel(x, edge_index, W1, b1, W2, b2):
    global LAST_RESULTS

    x = np.asarray(x, dtype=np.float32)
    edge_index = np.asarray(edge_index)
    W1 = np.asarray(W1, dtype=np.float32)
    b1 = np.asarray(b1, dtype=np.float32)
    W2 = np.asarray(W2, dtype=np.float32)
    b2 = np.asarray(b2, dtype=np.float32)

    src = edge_index[0].astype(np.int64)
    dst = edge_index[1].astype(np.int64)

    # --- host packing (index preprocessing + layout/dtype shuffles) ---
    x_rows = np.zeros((KSRC, HIDDEN), dtype=BF16)
    x_rows[:N_NODES] = x

    # Edge list incl. self-loops (the GIN (1+eps)*x_i term, eps=0), bucketed
    # by (core, dst-block of 128) and padded to T*128 per bucket with edges
    # from the all-zeros row.
    allsrc = np.concatenate([src, np.arange(N_NODES, dtype=np.int64)])
    alldst = np.concatenate([dst, np.arange(N_NODES, dtype=np.int64)])
    core = alldst // ND
    local = alldst % ND
    blk = local // P
    within = (local % P).astype(np.int32)
    bucket = core * NBLK + blk
    order = np.argsort(bucket, kind="stable")
    bs = bucket[order]
    counts = np.bincount(bucket, minlength=NCORES * NBLK)
    T = T_TILES
    maxc = int(counts.max())
    if maxc > T * P:
        T = -(-maxc // P)       # fallback: recompile with a bigger T
    cap = T * P
    starts = np.zeros(NCORES * NBLK, dtype=np.int64)
    np.cumsum(counts[:-1], out=starts[1:])
    pos = np.arange(bs.size, dtype=np.int64) - starts[bs]
    src_pad = np.full((NCORES * NBLK, cap), ZERO_ROW, dtype=np.int32)
    dst_pad = np.zeros((NCORES * NBLK, cap), dtype=np.float32)
    src_pad[bs, pos] = allsrc[order].astype(np.int32)
    dst_pad[bs, pos] = within[order]
    # [ncores*nblk, T*P] -> [ncores, nblk, P, T] (tile-major -> [P, T] slabs)
    src_pad = src_pad.reshape(NCORES, NBLK, T, P).transpose(0, 1, 3, 2)
    dst_pad = dst_pad.reshape(NCORES, NBLK, T, P).transpose(0, 1, 3, 2)

    colidx = np.broadcast_to(np.arange(P, dtype=np.float32)[None, :], (P, P))
    colidx = np.ascontiguousarray(colidx)
    w1t_kxm = _pack(W1.T.astype(BF16))                      # [512, 4352]
    w2t_kxm = _pack(W2.T.astype(BF16))                      # [4352, 8192]
    b1_mxn = _pack(np.broadcast_to(b1.astype(BF16)[:, None], (MIDDLE, NDP)))
    b2_mxn = _pack(np.broadcast_to(b2.astype(BF16)[:, None], (VOCAB, NDP)))

    in_maps = []
    for c in range(NCORES):
        in_maps.append({
            "x_rows": x_rows,
            "src_ids": np.ascontiguousarray(src_pad[c]),
            "dst_ids": np.ascontiguousarray(dst_pad[c]),
            "colidx": colidx,
            "w1t_kxm": w1t_kxm,
            "b1_mxn": b1_mxn,
            "w2t_kxm": w2t_kxm,
            "b2_mxn": b2_mxn,
        })

    ins_dev, zeros = _prep_device_inputs(in_maps, T)
    outs = _run_once(ins_dev, zeros, T)
    LAST_RESULTS = dict(ins_dev=ins_dev, outs=outs, T=T)

    r = _make_runner(T)
    out_global = np.asarray(outs[r["out_names"].index("out_mxn")])
    out_global = out_global.reshape(NCORES, P, VOCAB // P, NDP)

    out = np.empty((N_NODES, VOCAB), dtype=np.float32)
    for c in range(NCORES):
        o = _unpack(out_global[c])                          # [8192, 2560]
        out[c * ND:(c + 1) * ND] = o[:, :ND].T
    return out


def bench(iters=5):
    """Steady-state per-call wall time (s); requires kernel() to have run."""
    import time
    import jax
    st = LAST_RESULTS
    assert st is not None, "run kernel() first"
    outs = st["outs"]
    times = []
    for _ in range(iters):
        t0 = time.perf_counter()
        outs = _run_once(st["ins_dev"], outs, st["T"])
        times.append(time.perf_counter() - t0)
    st["outs"] = outs
    return times


def bench_pipelined(iters=8):
    """Dispatch `iters` chained calls without blocking, block once.

    Successive calls are serialized on-device by the donated-output data
    dependence, while host dispatch overlaps — the per-iter slope is the
    device execution time.
    """
    import time
    import jax
    st = LAST_RESULTS
    assert st is not None, "run kernel() first"
    r = _make_runner(st["T"])
    outs = st["outs"]
    # warm: one blocked call so everything is resident
    outs = _run_once(st["ins_dev"], outs, st["T"])
    t0 = time.perf_counter()
    outs = r["fn"](*st["ins_dev"], *outs)
    jax.block_until_ready(outs)
    t1 = time.perf_counter() - t0
    t0 = time.perf_counter()
    for _ in range(iters):
        outs = r["fn"](*st["ins_dev"], *outs)
    jax.block_until_ready(outs)
    tN = time.perf_counter() - t0
    st["outs"] = outs
    per_iter = (tN - t1) / (iters - 1)
    return dict(t1=t1, tN=tN, iters=iters, per_iter=per_iter)
